# revision 8
# baseline (speedup 1.0000x reference)
"""MLA (Multi-Head Latent Attention) Trainium2 kernel, 8-core SPMD.

Sharding: core c = (b, g): batch b = c//4, head-group g = c%4 (4 heads each).
Per core: full attention for its 4 heads on its batch; partial output
projection (summed on host over the 4 groups of each batch).

Layouts (device): activations kept transposed (features on partitions,
T on free) so every matmul contraction lands on partitions.
All matmuls in float32r (TF32-like, full PE speed at N>=256) except the
PV matmul (bf16 probs/values).

Softmax: no max-subtraction (scores are O(4) for this problem's scales);
rowsum via ACT Exp accum_out, normalization via per-partition tensor_scalar.
"""
import math
import numpy as np

import concourse.bass as bass
import concourse.bacc as bacc
import concourse.mybir as mybir
import concourse.tile as tile
from concourse.bass_utils import run_bass_kernel_spmd

F32 = mybir.dt.float32
F32R = mybir.dt.float32r
BF16 = mybir.dt.bfloat16
AF = mybir.ActivationFunctionType

H, D_MODEL, D_C, D_ROPE = 16, 2048, 512, 64
D_HEAD = D_MODEL // H          # 128
D_QK = D_HEAD + D_ROPE         # 192
N_CORES = 8
H_LOC = 4                      # heads per core
GW = H_LOC * D_HEAD            # 512: per-core width of W_Q/W_UK/W_UV/W_O
RW = H_LOC * D_ROPE            # 256: per-core width of W_QR
SCALE = 1.0 / math.sqrt(D_QK)

P = 128
KT_DM = D_MODEL // P           # 16 contraction tiles over d_model
KT_DC = D_C // P               # 4  contraction tiles over d_c
TCH = 256                      # phase-A T-chunk (moving free dim)
QC = 512                       # phase-B q-chunk


def _load_kxn(nc, dst_tile, dram, K, N):
    """DRAM [K, N] -> SBUF tile [128, (K//128)*N] (k-tile-major blocks)."""
    src = dram.ap().rearrange("(kt p) n -> p kt n", p=P)
    dst = dst_tile[:].rearrange("p (kt n) -> p kt n", n=N)
    nc.sync.dma_start(dst, src)


def build(T=2048):
    KT_T = T // P              # k/q tiles over sequence
    NCH = T // TCH             # phase-A chunks
    NQC = T // QC              # phase-B q-chunks
    NKC = T // 512             # 512-wide k chunks for S_nat

    nc = bacc.Bacc("TRN2", target_bir_lowering=False, debug=False,
                   num_devices=N_CORES)

    # ---- I/O ----
    xqT = nc.dram_tensor("xqT", [D_MODEL, T], F32R, kind="ExternalInput")
    xkT = nc.dram_tensor("xkT", [D_MODEL, T], F32R, kind="ExternalInput")
    wdkv = nc.dram_tensor("wdkv", [D_MODEL, D_C], F32R, kind="ExternalInput")
    wkr = nc.dram_tensor("wkr", [D_MODEL, D_ROPE], F32R, kind="ExternalInput")
    wq = nc.dram_tensor("wq", [D_MODEL, GW], F32R, kind="ExternalInput")
    wqr = nc.dram_tensor("wqr", [D_MODEL, RW], F32R, kind="ExternalInput")
    wuk = nc.dram_tensor("wuk", [D_C, GW], F32R, kind="ExternalInput")
    wuv = nc.dram_tensor("wuv", [D_C, GW], F32R, kind="ExternalInput")
    wo = nc.dram_tensor("wo", [GW, D_MODEL], F32R, kind="ExternalInput")
    cos2 = nc.dram_tensor("cos2", [P, T], F32R, kind="ExternalInput")
    sin2 = nc.dram_tensor("sin2", [P, T], F32R, kind="ExternalInput")
    perm = nc.dram_tensor("perm", [P, P], F32R, kind="ExternalInput")
    iden = nc.dram_tensor("iden", [P, P], F32R, kind="ExternalInput")
    p_out = nc.dram_tensor("p_out", [H_LOC, T, T], F32, kind="ExternalOutput")
    o_out = nc.dram_tensor("o_out", [T, D_MODEL], F32, kind="ExternalOutput")

    with tile.TileContext(nc) as tc:
        const = tc.alloc_tile_pool(name="const", bufs=1)
        persist = tc.alloc_tile_pool(name="persist", bufs=1)

        ident = const.tile([P, P], F32R)
        nc.sync.dma_start(ident[:], iden[:])
        perm_t = const.tile([P, P], F32R)
        nc.sync.dma_start(perm_t[:], perm[:])

        # persistent activations
        ckvT = persist.tile([P, KT_DC * T], F32R, tag="ckv")      # [d_c, T]
        qnT = persist.tile([P, H_LOC * T], F32R, tag="qn")        # [4*128, T]
        qrT = persist.tile([P, 2 * T], F32R, tag="qr")            # [2*128, T]
        krT = persist.tile([P, T], F32R, tag="kr")  # [128, T]: k_rope duplicated in both halves

        # ---------------- Phase A: projections + rope ----------------
        with tc.tile_pool(name="pa_sb", bufs=2) as pa_sb, \
             tc.tile_pool(name="pa_w", bufs=1) as pa_w, \
             tc.tile_pool(name="pa_ps", bufs=2, space="PSUM") as pa_ps:

            cos_t = pa_w.tile([P, T], F32R, tag="cos")
            sin_t = pa_w.tile([P, T], F32R, tag="sin")
            nc.sync.dma_start(cos_t[:], cos2[:])
            nc.sync.dma_start(sin_t[:], sin2[:])

            # --- A1: c_kvT, k_rope from xkT ---
            wdkv_t = pa_w.tile([P, KT_DM * D_C], F32R, tag="wdkv")
            _load_kxn(nc, wdkv_t, wdkv, D_MODEL, D_C)
            wkr_t = pa_w.tile([P, KT_DM * D_ROPE], F32R, tag="wkr")
            _load_kxn(nc, wkr_t, wkr, D_MODEL, D_ROPE)

            kr_raw = pa_sb.tile([64, T], F32R, tag="kr_raw", bufs=1)
            for ch in range(NCH):
                xc = pa_sb.tile([P, KT_DM * TCH], F32R, tag="xchunk")
                src = xkT.ap()[:, ch * TCH:(ch + 1) * TCH] \
                    .rearrange("(kt p) n -> p kt n", p=P)
                nc.sync.dma_start(
                    xc[:].rearrange("p (kt n) -> p kt n", n=TCH), src)
                # c_kvT chunk: 4 m-tiles
                for m in range(KT_DC):
                    ps = pa_ps.tile([P, TCH], F32, tag="proj")
                    for k in range(KT_DM):
                        nc.tensor.matmul(
                            ps[:], wdkv_t[:, k * D_C + m * P:k * D_C + (m + 1) * P],
                            xc[:, k * TCH:(k + 1) * TCH],
                            start=(k == 0), stop=(k == KT_DM - 1))
                    nc.vector.tensor_copy(
                        ckvT[:, m * T + ch * TCH:m * T + (ch + 1) * TCH], ps[:])
                # k_rope chunk
                ps = pa_ps.tile([64, TCH], F32, tag="rope_proj")
                for k in range(KT_DM):
                    nc.tensor.matmul(
                        ps[:], wkr_t[:, k * D_ROPE:(k + 1) * D_ROPE],
                        xc[:, k * TCH:(k + 1) * TCH],
                        start=(k == 0), stop=(k == KT_DM - 1))
                nc.vector.tensor_copy(
                    kr_raw[:, ch * TCH:(ch + 1) * TCH], ps[:])

            # rope on k: krT = kr_raw*cos + (perm @ kr_raw)*sin
            for ch in range(NCH):
                sl = slice(ch * TCH, (ch + 1) * TCH)
                pst = pa_ps.tile([64, TCH], F32, tag="rope_tmp")
                nc.tensor.matmul(pst[:], perm_t[0:64, 0:64], kr_raw[:, sl],
                                 start=True, stop=True)
                t1 = pa_sb.tile([64, TCH], F32R, tag="rope_t1")
                nc.vector.tensor_mul(t1[:], kr_raw[:, sl], cos_t[0:64, sl])
                t2 = pa_sb.tile([64, TCH], F32R, tag="rope_t2")
                nc.vector.tensor_mul(t2[:], pst[:], sin_t[0:64, sl])
                nc.vector.tensor_add(krT[0:64, sl], t1[:], t2[:])

            # duplicate k_rope rows into partitions 64:128 (PE needs operand
            # partition alignment; odd heads' q_rope lives at offset 64)
            nc.sync.dma_start(krT[64:128, :], krT[0:64, :])

            # --- A2: q projections from xqT ---
            wq_t = pa_w.tile([P, KT_DM * GW], F32R, tag="wdkv")  # reuse slot
            _load_kxn(nc, wq_t, wq, D_MODEL, GW)
            wqr_t = pa_w.tile([P, KT_DM * RW], F32R, tag="wqr")
            _load_kxn(nc, wqr_t, wqr, D_MODEL, RW)

            qr_raw = pa_sb.tile([P, 2 * T], F32R, tag="qr_raw", bufs=1)
            for ch in range(NCH):
                xc = pa_sb.tile([P, KT_DM * TCH], F32R, tag="xchunk")
                src = xqT.ap()[:, ch * TCH:(ch + 1) * TCH] \
                    .rearrange("(kt p) n -> p kt n", p=P)
                nc.sync.dma_start(
                    xc[:].rearrange("p (kt n) -> p kt n", n=TCH), src)
                for m in range(H_LOC):
                    ps = pa_ps.tile([P, TCH], F32, tag="proj")
                    for k in range(KT_DM):
                        nc.tensor.matmul(
                            ps[:], wq_t[:, k * GW + m * P:k * GW + (m + 1) * P],
                            xc[:, k * TCH:(k + 1) * TCH],
                            start=(k == 0), stop=(k == KT_DM - 1))
                    nc.vector.tensor_copy(
                        qnT[:, m * T + ch * TCH:m * T + (ch + 1) * TCH], ps[:])
                for m in range(2):
                    ps = pa_ps.tile([P, TCH], F32, tag="proj")
                    for k in range(KT_DM):
                        nc.tensor.matmul(
                            ps[:], wqr_t[:, k * RW + m * P:k * RW + (m + 1) * P],
                            xc[:, k * TCH:(k + 1) * TCH],
                            start=(k == 0), stop=(k == KT_DM - 1))
                    nc.vector.tensor_copy(
                        qr_raw[:, m * T + ch * TCH:m * T + (ch + 1) * TCH], ps[:])

            # rope on q (two 128-row tiles, each = 2 heads)
            for m in range(2):
                for ch in range(NCH):
                    sl = slice(ch * TCH, (ch + 1) * TCH)
                    msl = slice(m * T + ch * TCH, m * T + (ch + 1) * TCH)
                    pst = pa_ps.tile([P, TCH], F32, tag="rope_tmp")
                    nc.tensor.matmul(pst[:], perm_t[:], qr_raw[:, msl],
                                     start=True, stop=True)
                    t1 = pa_sb.tile([P, TCH], F32R, tag="rope_t1")
                    nc.vector.tensor_mul(t1[:], qr_raw[:, msl], cos_t[:, sl])
                    t2 = pa_sb.tile([P, TCH], F32R, tag="rope_t2")
                    nc.vector.tensor_mul(t2[:], pst[:], sin_t[:, sl])
                    nc.vector.tensor_add(qrT[:, msl], t1[:], t2[:])

        # ---------------- Phase B: attention per head ----------------
        xpool = tc.alloc_tile_pool(name="xpool", bufs=1)
        x_h = []
        for h in range(H_LOC):
            xh = xpool.tile([P, T], F32R, tag=f"xh{h}", name=f"xh{h}")
            x_h.append(xh)

        with tc.tile_pool(name="pb_w", bufs=1) as pb_w, \
             tc.tile_pool(name="pb_v", bufs=1) as pb_v, \
             tc.tile_pool(name="pb_kh", bufs=1) as pb_kh, \
             tc.tile_pool(name="pb_es", bufs=2) as pb_es, \
             tc.tile_pool(name="pb_pst", bufs=2) as pb_pst, \
             tc.tile_pool(name="pb_sm", bufs=3) as pb_sm, \
             tc.tile_pool(name="pb_ps", bufs=2, space="PSUM") as pb_ps:

            wuk_t = pb_w.tile([P, KT_DC * GW], F32R, tag="wuk")
            _load_kxn(nc, wuk_t, wuk, D_C, GW)
            wuv_t = pb_w.tile([P, KT_DC * GW], F32R, tag="wuv")
            _load_kxn(nc, wuv_t, wuv, D_C, GW)

            # B0: v for all heads, natural layout, bf16.
            # v tile blocks: per k-tile i: [4 heads * 128] columns.
            v_t = pb_v.tile([P, KT_T * GW], BF16, tag="v")
            for i in range(KT_T):
                ps = pb_ps.tile([P, GW], F32, tag="mm512", bufs=3)
                for k in range(KT_DC):
                    nc.tensor.matmul(
                        ps[:], ckvT[:, k * T + i * P:k * T + (i + 1) * P],
                        wuv_t[:, k * GW:(k + 1) * GW],
                        start=(k == 0), stop=(k == KT_DC - 1))
                nc.vector.tensor_copy(v_t[:, i * GW:(i + 1) * GW], ps[:])

            for h in range(H_LOC):
                # k_nopeT for this head: [128, T]
                kh = pb_kh.tile([P, T], F32R, tag="kh", bufs=2)
                for c in range(NKC):
                    ps = pb_ps.tile([P, 512], F32, tag="mm512", bufs=3)
                    for k in range(KT_DC):
                        nc.tensor.matmul(
                            ps[:],
                            wuk_t[:, k * GW + h * P:k * GW + (h + 1) * P],
                            ckvT[:, k * T + c * 512:k * T + (c + 1) * 512],
                            start=(k == 0), stop=(k == KT_DC - 1))
                    nc.vector.tensor_copy(kh[:, c * 512:(c + 1) * 512], ps[:])

                qr_po = (h % 2) * 64          # partition offset in qrT
                qr_mo = (h // 2) * T          # m-tile column offset in qrT

                for qc in range(NQC):
                    # S_T -> expS_T (bf16) for this q-chunk
                    es = pb_es.tile([P, KT_T * QC], BF16, tag="es")
                    for kt in range(KT_T):
                        ps = pb_ps.tile([P, QC], F32, tag="mm512", bufs=3)
                        nc.tensor.matmul(
                            ps[:], kh[:, kt * P:(kt + 1) * P],
                            qnT[:, h * T + qc * QC:h * T + (qc + 1) * QC],
                            start=True, stop=False)
                        nc.tensor.matmul(
                            ps[:], krT[qr_po:qr_po + 64, kt * P:(kt + 1) * P],
                            qrT[qr_po:qr_po + 64,
                                qr_mo + qc * QC:qr_mo + (qc + 1) * QC],
                            start=False, stop=True)
                        nc.scalar.activation(es[:, kt * QC:(kt + 1) * QC],
                                             ps[:], AF.Exp, scale=SCALE)

                    for ql in range(QC // P):
                        qt = qc * (QC // P) + ql
                        # S_nat -> normalized p rows
                        pstage = pb_pst.tile([P, T], F32, tag="pstage")
                        racc = pb_sm.tile([P, NKC], F32, tag="racc")
                        for c in range(NKC):
                            ps = pb_ps.tile([P, 512], F32, tag="mm512", bufs=3)
                            nc.tensor.matmul(
                                ps[:],
                                qnT[:, h * T + qt * P:h * T + (qt + 1) * P],
                                kh[:, c * 512:(c + 1) * 512],
                                start=True, stop=False)
                            nc.tensor.matmul(
                                ps[:],
                                qrT[qr_po:qr_po + 64,
                                    qr_mo + qt * P:qr_mo + (qt + 1) * P],
                                krT[qr_po:qr_po + 64, c * 512:(c + 1) * 512],
                                start=False, stop=True)
                            nc.scalar.activation(
                                pstage[:, c * 512:(c + 1) * 512], ps[:],
                                AF.Exp, scale=SCALE,
                                accum_out=racc[:, c:c + 1])
                        rsum = pb_sm.tile([P, 1], F32, tag="rsum")
                        nc.vector.reduce_sum(rsum[:], racc[:],
                                             axis=mybir.AxisListType.X)
                        r = pb_sm.tile([P, 1], F32, tag="r")
                        nc.vector.reciprocal(r[:], rsum[:])
                        nc.vector.tensor_scalar_mul(pstage[:], pstage[:], r[:])
                        nc.sync.dma_start(
                            p_out.ap()[h, qt * P:(qt + 1) * P, :], pstage[:])

                        # PV: x_nat [q,d] then transpose into x_h
                        psx = pb_ps.tile([P, P], F32, tag="x", bufs=2)
                        for kt in range(KT_T):
                            nc.tensor.matmul(
                                psx[:],
                                es[:, kt * QC + ql * P:kt * QC + (ql + 1) * P],
                                v_t[:, kt * GW + h * P:kt * GW + (h + 1) * P],
                                start=(kt == 0), stop=(kt == KT_T - 1))
                        xn = pb_sm.tile([P, P], F32R, tag="xn")
                        nc.vector.tensor_scalar_mul(xn[:], psx[:], r[:])
                        psxt = pb_ps.tile([P, P], F32R, tag="xt", bufs=2)
                        nc.tensor.transpose(psxt[:], xn[:], ident[:])
                        nc.vector.tensor_copy(x_h[h][:, qt * P:(qt + 1) * P],
                                              psxt[:])

        # ---------------- Phase C: output projection ----------------
        with tc.tile_pool(name="pc_w", bufs=1) as pc_w, \
             tc.tile_pool(name="pc_sb", bufs=3) as pc_sb, \
             tc.tile_pool(name="pc_ps", bufs=2, space="PSUM") as pc_ps:
            wo_t = pc_w.tile([P, H_LOC * D_MODEL], F32R, tag="wo")
            _load_kxn(nc, wo_t, wo, GW, D_MODEL)
            for i in range(KT_T):
                for c in range(4):
                    ps = pc_ps.tile([P, 512], F32, tag="o")
                    for h in range(H_LOC):
                        nc.tensor.matmul(
                            ps[:], x_h[h][:, i * P:(i + 1) * P],
                            wo_t[:, h * D_MODEL + c * 512:
                                 h * D_MODEL + (c + 1) * 512],
                            start=(h == 0), stop=(h == H_LOC - 1))
                    ot = pc_sb.tile([P, 512], F32, tag="ostage")
                    nc.scalar.activation(ot[:], ps[:], AF.Copy)
                    nc.sync.dma_start(
                        o_out.ap()[i * P:(i + 1) * P, c * 512:(c + 1) * 512],
                        ot[:])

        xpool.release()
        persist.release()
        const.release()

    nc.compile()
    return nc


def _host_tables(T):
    inv_freq = 1.0 / (10000.0 ** (np.arange(0, D_ROPE, 2, dtype=np.float32)
                                  / D_ROPE))
    t = np.arange(T, dtype=np.float32)
    ang = np.outer(t, inv_freq)                      # [T, 32]
    cos = np.cos(ang).astype(np.float32)
    sin = np.sin(ang).astype(np.float32)
    # expand to [64, T]: row d -> pair d//2, stacked twice -> [128, T]
    cos64 = np.repeat(cos.T, 2, axis=0)              # [64, T]
    sin64 = np.repeat(sin.T, 2, axis=0)
    cos2 = np.concatenate([cos64, cos64], axis=0)    # [128, T]
    sin2 = np.concatenate([sin64, sin64], axis=0)
    # perm for interleaved rope: out[2i] = -x[2i+1], out[2i+1] = x[2i]
    # matmul form: out[m] = sum_k lhsT[k, m] x[k]
    perm64 = np.zeros((64, 64), dtype=np.float32)
    for i in range(32):
        perm64[2 * i + 1, 2 * i] = -1.0
        perm64[2 * i, 2 * i + 1] = 1.0
    perm = np.zeros((128, 128), dtype=np.float32)
    perm[:64, :64] = perm64
    perm[64:, 64:] = perm64
    return cos2, sin2, perm


_NC_CACHE = {}
_last_in_maps = None


def _run(inputs, T):
    if T not in _NC_CACHE:
        _NC_CACHE[T] = build(T)
    nc = _NC_CACHE[T]

    query, key = inputs["query"], inputs["key"]
    B = query.shape[0]
    cos2, sin2, perm = _host_tables(T)
    xqT = [np.ascontiguousarray(np.asarray(query[b]).T) for b in range(B)]
    xkT = [np.ascontiguousarray(np.asarray(key[b]).T) for b in range(B)]
    W = {k: np.asarray(inputs[k]) for k in
         ("W_DKV", "W_UK", "W_UV", "W_Q", "W_KR", "W_QR", "W_O")}

    in_maps = []
    for c in range(N_CORES):
        b, g = c // 4, c % 4
        in_maps.append({
            "xqT": xqT[b], "xkT": xkT[b],
            "wdkv": W["W_DKV"], "wkr": W["W_KR"],
            "wq": np.ascontiguousarray(W["W_Q"][:, g * GW:(g + 1) * GW]),
            "wqr": np.ascontiguousarray(W["W_QR"][:, g * RW:(g + 1) * RW]),
            "wuk": np.ascontiguousarray(W["W_UK"][:, g * GW:(g + 1) * GW]),
            "wuv": np.ascontiguousarray(W["W_UV"][:, g * GW:(g + 1) * GW]),
            "wo": np.ascontiguousarray(W["W_O"][g * GW:(g + 1) * GW, :]),
            "cos2": cos2, "sin2": sin2, "perm": perm,
            "iden": np.eye(128, dtype=np.float32),
        })
    global _last_in_maps
    _last_in_maps = in_maps
    res = run_bass_kernel_spmd(nc, in_maps, list(range(N_CORES)))

    output = np.zeros((B, T, D_MODEL), np.float32)
    p_attn = np.empty((B, H, T, T), np.float32)
    for c in range(N_CORES):
        b, g = c // 4, c % 4
        output[b] += res.results[c]["o_out"]
        p_attn[b, H_LOC * g:H_LOC * (g + 1)] = res.results[c]["p_out"]
    return output, p_attn


def kernel(query, key, value, W_DKV, W_UK, W_UV, W_Q, W_KR, W_QR, W_O):
    inputs = dict(query=query, key=key, value=value, W_DKV=W_DKV, W_UK=W_UK,
                  W_UV=W_UV, W_Q=W_Q, W_KR=W_KR, W_QR=W_QR, W_O=W_O)
    T = np.asarray(query).shape[1]
    return _run(inputs, T)
